# revision 53
# baseline (speedup 1.0000x reference)
"""Trainium2 Bass kernel for causal MHA (B=4, L=2048, D=1024, H=16), 8 cores.

Sharding: data-parallel over batch (4) x tensor-parallel over heads (2).
Each core handles one batch element and 8 heads (4 pairs).

Structure (v2 — software-pipelined around the Activation engine):
The attention phase is ACT(exp)-bound (~1.1us per key-block group of
[128, 2, 512] exp), so the kernel is organized to make everything else
disappear inside that window:
  - DMA: priority-ordered ~256KB transfers across the sync/scalar/gpsimd
    queues; weights stored e-tile-major on host so every transfer has
    2KB contiguous lines. Only wv + x-chunk0 + wqk e-tiles 0/1 gate the
    start; first matmuls begin ~4us after the queues spin up.
  - Pre-phase: V-proj kb0-3 + QK e-tiles 0/1 chunk 0 only (~12us), then
    attention pair 0 starts immediately.
  - All remaining projections (V kb4-15, QK e-tiles for later chunks and
    pairs, out-proj) are PE "fillers" with per-chain deadlines derived
    from the attention group that consumes them; a deadline+rate
    scheduler paces them into the per-group PE slack under the exp.
  - exp is batched across the two heads of a pair ([P, 2, 512] per
    ACTIVATE); score matmuls run one key-block ahead of the AV matmuls
    so AV never parks the PE on the exp that feeds it; the Scalar queue
    does nothing but exp during attention (PSUM drains on DVE).
  - softmax normalization for token-chunk c is deferred into chunk c+1
    (denominator rows -> DMA transpose -> reciprocal -> DMA back -> one
    K=2 selector-matmul broadcast); the final chunk uses a fast path
    with direct reciprocals and K=1 broadcasts.
  - out is written bf16 (host upcasts + sums the TP pairs in f32).
"""

import collections
import contextlib

import numpy as np

import concourse.bass as bass
import concourse.bacc as bacc
import concourse.mybir as mybir
import concourse.tile as tile

P = 128
HD = 64  # head dim
HOIST_CLOSE = True  # bisect flag: defer chunk close into the next chunk

F32 = mybir.dt.float32
BF16 = mybir.dt.bfloat16


def build_mha_nc(L, D, HEADS):
    """Build the per-core Bass program (HEADS = heads per core)."""
    DBLK = D // P          # contraction blocks for projections
    KB = L // P            # key blocks
    MC = L // 512          # token chunks for projections
    EQK = 2 * HEADS * HD   # q+k output channels per core
    ET = EQK // P          # qk e-tiles (q/k pair-interleaved)
    EV = HEADS * HD        # v output channels per core
    PAIRS = HEADS // 2
    QS = min(512, L)       # q-span per AV-psum accumulation
    NQ = L // QS
    RPH = QS // P          # denominator rows per (chunk, head)
    assert L % 512 == 0 and D % P == 0 and EV % P == 0 and HEADS % 2 == 0

    nc = bacc.Bacc("TRN2", target_bir_lowering=False, debug=False,
                   enable_asserts=False)

    xT = nc.dram_tensor("xT", [D, L], BF16, kind="ExternalInput").ap()
    # e-tile-major weights: per partition a contiguous [n_et, DBLK, P] block
    wqkT = nc.dram_tensor("wqkT", [P, ET, DBLK, P], BF16,
                          kind="ExternalInput").ap()
    wvT = nc.dram_tensor("wvT", [P, DBLK, EV], BF16,
                         kind="ExternalInput").ap()
    bqk = nc.dram_tensor("bqk", [P, ET], F32, kind="ExternalInput").ap()
    vb = nc.dram_tensor("vb", [P, EV], BF16, kind="ExternalInput").ap()
    woT = nc.dram_tensor("woT", [EV, D], BF16, kind="ExternalInput").ap()
    ob = nc.dram_tensor("ob", [P, D], BF16, kind="ExternalInput").ap()
    tri = nc.dram_tensor("tri", [P, P], BF16, kind="ExternalInput").ap()
    onec = nc.dram_tensor("onec", [P, KB], BF16, kind="ExternalInput").ap()
    sel2 = nc.dram_tensor("sel2", [2, P], BF16, kind="ExternalInput").ap()
    onep = nc.dram_tensor("onep", [P, HD], BF16, kind="ExternalInput").ap()
    out = nc.dram_tensor("out", [L, D], BF16, kind="ExternalOutput").ap()

    scale = 1.0 / float(np.sqrt(HD))

    with tile.TileContext(nc) as tc:
        ctx = contextlib.ExitStack()
        with ctx:
            consts = ctx.enter_context(tc.tile_pool(name="consts", bufs=1))
            wqk_pool = ctx.enter_context(tc.tile_pool(name="wqk", bufs=1))
            wvo_pool = ctx.enter_context(tc.tile_pool(name="wvo", bufs=1))
            xt_pool = ctx.enter_context(tc.tile_pool(name="xt", bufs=MC))
            qk_pool = ctx.enter_context(tc.tile_pool(name="qk", bufs=ET))
            vst_pool = ctx.enter_context(tc.tile_pool(name="vst", bufs=KB))
            ex_pool = ctx.enter_context(tc.tile_pool(name="ex", bufs=8))
            attn_pool = ctx.enter_context(tc.tile_pool(name="attn", bufs=1))
            outst_pool = ctx.enter_context(tc.tile_pool(name="outst", bufs=6))
            den_pool = ctx.enter_context(tc.tile_pool(name="den", bufs=2))
            recl_pool = ctx.enter_context(tc.tile_pool(name="recl", bufs=2))
            drow_pool = ctx.enter_context(tc.tile_pool(name="drow", bufs=2))
            tmp_pool = ctx.enter_context(tc.tile_pool(name="tmp", bufs=3))
            st_ps = ctx.enter_context(
                tc.tile_pool(name="st_ps", bufs=2, space="PSUM"))
            av_ps = ctx.enter_context(
                tc.tile_pool(name="av_ps", bufs=2, space="PSUM"))
            mm_ps = ctx.enter_context(
                tc.tile_pool(name="mm_ps", bufs=2, space="PSUM"))

            # ================= DMA (priority-ordered) =================
            # sync is the fastest queue, scalar next, gpsimd (software
            # DGE) slowest. scalar must be clean once attention starts
            # (it is the ACT queue), so everything on it is front-loaded.
            wv_sb = wvo_pool.tile([P, DBLK, EV], BF16, name="wv_sb",
                                  tag="wvo")
            xt_tiles = [xt_pool.tile([P, DBLK, 512], BF16, name=f"xt_{mc}",
                                     tag="xt") for mc in range(MC)]
            wqk_sb = wqk_pool.tile([P, ET, DBLK, P], BF16, name="wqk_sb")
            xT_blocked = xT.rearrange("(o p) m -> p o m", p=P)

            def dma_xt(eng, mc, o0, o1):
                eng.dma_start(out=xt_tiles[mc][:, o0:o1, :],
                              in_=xT_blocked[:, o0:o1,
                                             mc * 512:(mc + 1) * 512])

            # Queues drain strictly serially (ring flow control), so each
            # queue's emission order IS its arrival order. The attention
            # gate (xt chunk0 + wqk e-tiles 0/1) goes first.
            # -- sync: x chunk0, V weights, x chunk1, wo halves, x tails --
            dma_xt(nc.sync, 0, 0, 2)
            dma_xt(nc.sync, 0, 2, 4)
            dma_xt(nc.sync, 0, 4, 6)
            nc.sync.dma_start(out=wv_sb[:, 0:DBLK // 2, :],
                              in_=wvT[:, 0:DBLK // 2, :])
            dma_xt(nc.sync, 1, 0, 4)
            dma_xt(nc.sync, 1, 4, 8)
            wo_sb = wvo_pool.tile([P, EV // P, D], BF16, name="wo_sb",
                                  tag="wo")
            _wo_src = woT.rearrange("(j p) f -> p j f", p=P)
            nc.sync.dma_start(out=wo_sb[:, 0:1, :], in_=_wo_src[:, 0:1, :])
            dma_xt(nc.sync, 2, 4, 8)
            nc.sync.dma_start(out=wo_sb[:, 2:3, :], in_=_wo_src[:, 2:3, :])
            dma_xt(nc.sync, 3, 4, 8)
            # -- scalar: qk weights then x middles; clean before exps --
            for et in (0, 1):
                nc.scalar.dma_start(out=wqk_sb[:, et], in_=wqkT[:, et])
            nc.scalar.dma_start(out=wv_sb[:, DBLK // 2:DBLK, :],
                                in_=wvT[:, DBLK // 2:DBLK, :])
            for et in (2, 3):
                nc.scalar.dma_start(out=wqk_sb[:, et], in_=wqkT[:, et])
            dma_xt(nc.scalar, 2, 0, 4)
            dma_xt(nc.scalar, 3, 0, 4)
            for et in (4, 5, 6, 7):
                nc.scalar.dma_start(out=wqk_sb[:, et], in_=wqkT[:, et])
            # -- gpsimd (slow): x chunk0 tail, consts, deferrable bulk --
            dma_xt(nc.gpsimd, 0, 6, 8)
            bqk_sb = consts.tile([P, ET], F32, name="bqk_sb")
            nc.gpsimd.dma_start(out=bqk_sb, in_=bqk)
            tri_sb = consts.tile([P, P], BF16, name="tri_sb")
            nc.gpsimd.dma_start(out=tri_sb, in_=tri)
            ones_c = consts.tile([P, KB], BF16, name="ones_c")
            nc.gpsimd.dma_start(out=ones_c, in_=onec)
            vb_sb = consts.tile([P, EV], BF16, name="vb_sb")
            nc.gpsimd.dma_start(out=vb_sb, in_=vb)
            sel2_sb = consts.tile([2, P], BF16, name="sel2_sb")
            nc.gpsimd.dma_start(out=sel2_sb, in_=sel2)
            onep_sb = consts.tile([P, HD], BF16, name="onep_sb")
            nc.gpsimd.dma_start(out=onep_sb, in_=onep)
            ob_sb = consts.tile([P, D], BF16, name="ob_sb")
            nc.gpsimd.dma_start(out=ob_sb, in_=ob)
            nc.gpsimd.dma_start(out=wo_sb[:, 1:2, :], in_=_wo_src[:, 1:2, :])
            nc.gpsimd.dma_start(out=wo_sb[:, 3:4, :], in_=_wo_src[:, 3:4, :])
            tri_bc = tri_sb.unsqueeze(1).broadcast_to((P, 2, P))

            # ================= projection chain builders =================
            vb_h = vb_sb.rearrange("p (h c) -> p h c", c=HD)
            vst_tiles = [vst_pool.tile([P, HEADS, HD + 1], BF16,
                                       name=f"vst_{kb}", tag="vst")
                         for kb in range(KB)]

            def v_chain_parts(kb, nsplit=4):
                """V projection for key-block kb, split for pacing."""
                mc, mt = kb // 4, kb % 4
                state = {}
                per = DBLK // nsplit
                def part(i):
                    def go():
                        if i == 0:
                            state["ps"] = mm_ps.tile(
                                [P, 512], F32, name=f"vps_{kb}", tag="mm")
                        ps = state["ps"]
                        for o in range(i * per, (i + 1) * per):
                            nc.tensor.matmul(
                                ps[:, 0:EV],
                                lhsT=xt_tiles[mc][:, o, mt * P:(mt + 1) * P],
                                rhs=wv_sb[:, o, :],
                                start=(o == 0), stop=(o == DBLK - 1))
                        if i == nsplit - 1:
                            vst = vst_tiles[kb]
                            nc.vector.tensor_add(
                                out=vst[:, :, 0:HD],
                                in0=ps[:, 0:EV].rearrange(
                                    "p (h c) -> p h c", c=HD),
                                in1=vb_h)
                            nc.vector.tensor_copy(
                                out=vst[:, :, HD:HD + 1],
                                in_=ones_c[:, 0:HEADS, None])
                    return go
                return [part(i) for i in range(nsplit)]

            qk_tiles = [qk_pool.tile([P, L], BF16, name=f"qk_{et}", tag="qk")
                        for et in range(ET)]

            def qk_chain_parts(et, mc, nsplit=4):
                state = {}
                per = DBLK // nsplit
                def part(i):
                    def go():
                        if i == 0:
                            state["ps"] = mm_ps.tile(
                                [P, 512], F32, name=f"qkps_{et}_{mc}",
                                tag="mm")
                        ps = state["ps"]
                        for o in range(i * per, (i + 1) * per):
                            nc.tensor.matmul(
                                ps,
                                lhsT=wqk_sb[:, et, o, :],
                                rhs=xt_tiles[mc][:, o, :],
                                start=(o == 0), stop=(o == DBLK - 1))
                        if i == nsplit - 1:
                            nc.vector.tensor_scalar(
                                out=qk_tiles[et][:, mc * 512:(mc + 1) * 512],
                                in0=ps, scalar1=bqk_sb[:, et:et + 1],
                                scalar2=None, op0=mybir.AluOpType.add)
                    return go
                return [part(i) for i in range(nsplit)]

            def out_chain_parts(qt, f0, fn, nsplit=2, pool=None, ptag=None):
                state = {}
                NJ = EV // P
                per = NJ // nsplit
                def part(i):
                    def go():
                        if i == 0:
                            state["ps"] = (pool or mm_ps).tile(
                                [P, 512], F32, name=f"ops_{qt}_{f0}",
                                tag=ptag or "mm")
                        ps = state["ps"]
                        for j in range(i * per, (i + 1) * per):
                            nc.tensor.matmul(
                                ps[:, 0:fn],
                                lhsT=attn_sb[:, j, qt * P:(qt + 1) * P],
                                rhs=wo_sb[:, j, f0:f0 + fn],
                                start=(j == 0), stop=(j == NJ - 1))
                        if i == nsplit - 1:
                            ot = outst_pool.tile(
                                [P, 512], BF16, name=f"ot_{qt}_{f0}",
                                tag="outst")
                            nc.vector.tensor_add(
                                out=ot[:, 0:fn], in0=ps[:, 0:fn],
                                in1=ob_sb[:, f0:f0 + fn])
                            # first token chunk rides the slow software
                            # queue; the final chunk splits across sync and
                            # the (by then idle) scalar queue; rest on sync
                            if qt < L // P // 4:
                                eng = nc.gpsimd
                            elif qt >= 3 * (L // P) // 4 and f0 >= 512:
                                eng = nc.scalar
                            else:
                                eng = nc.sync
                            eng.dma_start(
                                out=out[qt * P:(qt + 1) * P, f0:f0 + fn],
                                in_=ot[:, 0:fn])
                    return go
                return [part(i) for i in range(nsplit)]

            # ================= deadline + rate filler pacing =============
            # tick counter: 2 ticks per attention kb-group, accumulated
            # over the whole attention phase.
            def tick_of_chunk(pr, q4):
                return 2 * (pr * (NQ * (NQ + 1) * 2) +
                            2 * q4 * q4 + 2 * q4)

            TOTAL_TICKS = tick_of_chunk(PAIRS, 0) if PAIRS else 0

            class Fillers:
                """Queue of (deadline_tick, fn). Due fillers emit
                unconditionally; the rest are paced by rate."""
                def __init__(self):
                    self.q = collections.deque()
                    self.emitted = 0
                    self.ticks = 0
                    self.rate = 0.0

                def push(self, fn, deadline=None):
                    self.q.append((deadline, fn))

                def tick(self):
                    self.ticks += 1
                    self.due()
                    while self.q and self.emitted < self.rate * self.ticks:
                        self.q.popleft()[1]()
                        self.emitted += 1

                def due(self, ahead=0):
                    """Emit every filler whose deadline has arrived. Called
                    before each chunk's first scores as a hard guarantee
                    that producers are emitted before their consumers."""
                    while self.q and self.q[0][0] is not None \
                            and self.q[0][0] <= self.ticks + ahead:
                        self.q.popleft()[1]()
                        self.emitted += 1

                def drain(self):
                    while self.q:
                        self.q.popleft()[1]()

                def set_rate(self, rate):
                    self.rate = rate
                    self.emitted = 0
                    self.ticks = 0

            fillers = Fillers()

            # ---- pre-phase: only QK e-tiles 0/1 chunk 0 gate attention ----
            for et in (0, 1):
                for p_ in qk_chain_parts(et, mc=0, nsplit=1):
                    p_()

            # ---- everything else becomes deadline fillers ----
            # (deadline, seq, chain-part) collected then globally sorted
            work = []
            # V kb: consumed by AV(pair0, chunk kb//4, group kb%4+1)
            for kb in range(KB):
                dl = tick_of_chunk(0, kb // 4) + 2 * (kb % 4) - 2
                work.append((dl, v_chain_parts(kb)))
            # QK q-tile (even et) due at chunk start; k-tile (odd et) due
            # at the diagonal group of its chunk. The LAST pair's weights
            # are pulled into the previous pair so the last pair's PE
            # budget is reserved for the output projection.
            for mc in range(1, MC):
                work.append((tick_of_chunk(0, mc) - 12,
                             qk_chain_parts(0, mc)))
                work.append((tick_of_chunk(0, mc) + 8 * mc - 12,
                             qk_chain_parts(1, mc)))
            for pr in range(1, PAIRS):
                dpr = min(pr, PAIRS - 2)
                for mc in range(MC):
                    if pr == dpr:
                        # q needed at chunk start; k at its diagonal group
                        dlq = tick_of_chunk(pr, mc) - 12
                        dlk = tick_of_chunk(pr, mc) + 8 * mc - 12
                    else:
                        # last pair's weights land inside the previous pair
                        dlq = tick_of_chunk(dpr, mc) + 4 * mc + 8
                        dlk = dlq + 4
                    work.append((dlq, qk_chain_parts(2 * pr, mc)))
                    work.append((dlk, qk_chain_parts(2 * pr + 1, mc)))
            work.sort(key=lambda w: w[0])
            for dl, parts in work:
                for p_ in parts:
                    # never due before the first scores+exp are emitted
                    fillers.push(p_, deadline=max(dl, 2))
            # rate: spread everything across the first 3 pairs' ticks
            fillers.rate = len(fillers.q) / max(tick_of_chunk(3, 0), 1)

            attn_sb = attn_pool.tile([P, PAIRS, L], BF16, name="attn_sb")

            pending_norm = collections.deque()
            pending_close = collections.deque()

            def attention_pair(pr, last_pair):
                q_tile = qk_tiles[2 * pr]
                k_tile = qk_tiles[2 * pr + 1]
                den = den_pool.tile([2 * RPH, NQ, P], BF16,
                                    name=f"den_{pr}", tag="den")
                recl = recl_pool.tile([2, NQ, QS], BF16,
                                      name=f"recl_{pr}", tag="recl")
                for q4 in range(NQ):
                    q0 = q4 * QS
                    avs = []
                    last_kb = (q0 + QS) // P - 1
                    pending_avs = collections.deque()
                    # double-groups: emit scores+exp for two key-blocks
                    # back-to-back so the PE weight-generation swap around
                    # each concurrent score pair is paid once per TWO
                    # groups; AVs trail two groups behind
                    for kb2 in range(0, last_kb + 1, 2):
                        for kb in (kb2, kb2 + 1):
                            s0 = max(0, kb * P - q0)
                            d0 = kb * P - q0
                            if kb == 0:
                                # hard guarantee: every due producer is
                                # emitted before this chunk's consumers
                                fillers.due(1)
                            st = st_ps.tile([P, 2, QS], F32,
                                            name=f"st_{pr}_{q4}_{kb}",
                                            tag="st")
                            for hh in (0, 1):
                                rows = slice(hh * HD, hh * HD + HD)
                                nc.tensor.matmul(
                                    st[:, hh, s0:QS],
                                    lhsT=k_tile[rows, kb * P:(kb + 1) * P],
                                    rhs=q_tile[rows, q0 + s0:q0 + QS],
                                    start=True, stop=True)
                            ex = ex_pool.tile([P, 2, QS], BF16,
                                              name=f"ex_{pr}_{q4}_{kb}",
                                              tag="ex")
                            nc.scalar.activation(
                                out=ex[:, :, s0:QS], in_=st[:, :, s0:QS],
                                func=mybir.ActivationFunctionType.Exp,
                                scale=scale)
                            if d0 >= 0:
                                nc.vector.tensor_mul(
                                    out=ex[:, :, d0:d0 + P],
                                    in0=ex[:, :, d0:d0 + P], in1=tri_bc)
                            # previous chunk's last AVs + drains slot in
                            # right after this chunk's pipeline is primed;
                            # only then may its av psum slots be claimed
                            if kb == 0:
                                if pending_close:
                                    pending_close.popleft()()
                                avs.extend(
                                    av_ps.tile([HD + 1, QS], F32,
                                               name=f"av_{pr}_{q4}_{hh}",
                                               tag="av")
                                    for hh in (0, 1))
                            def make_av(kb=kb, s0=s0, ex=ex, avs=avs):
                                def go():
                                    for hh in (0, 1):
                                        nc.tensor.matmul(
                                            avs[hh][:, s0:QS],
                                            lhsT=vst_tiles[kb][
                                                :, 2 * pr + hh, :],
                                            rhs=ex[:, hh, s0:QS],
                                            start=(kb == 0),
                                            stop=(kb == last_kb))
                                return go
                            pending_avs.append(make_av())
                        fillers.tick()
                        while len(pending_avs) > 2:
                            pending_avs.popleft()()
                        fillers.tick()
                        pop_kb = min(4, last_kb - 1) if last_pair \
                            else max(1, last_kb - 1)
                        if pop_kb in (kb2, kb2 + 1) and pending_norm:
                            pending_norm.popleft()()
                        fillers.tick()
                        fillers.tick()

                    fin_avs = list(pending_avs)
                    pending_avs.clear()

                    # the final chunk's bounce DMAs ride the scalar queue
                    # (idle once the exps are done); everything else on sync
                    tail_chunk = last_pair and q4 == NQ - 1
                    beng = nc.scalar if tail_chunk else nc.sync

                    def close(pr=pr, q4=q4, q0=q0, avs=avs, den=den,
                              recl=recl, final_avs=fin_avs, beng=beng):
                        for f_ in final_avs:
                            f_()
                        # drain av psum (DVE only; keep ACT exp-clean)
                        dr = drow_pool.tile([1, QS], BF16,
                                            name=f"dr_{pr}_{q4}", tag="drow")
                        tmp = tmp_pool.tile([HD + 1, QS], BF16,
                                            name=f"tmp_{pr}_{q4}", tag="tmp")
                        nc.vector.tensor_copy(
                            out=attn_sb[0:HD, pr, q0:q0 + QS],
                            in_=avs[0][0:HD, :])
                        nc.vector.tensor_copy(out=dr, in_=avs[0][HD:HD + 1, :])
                        if beng is nc.scalar:
                            # tail: ACT is idle after the last exp — drain
                            # the odd head there, parallel to the DVE
                            nc.scalar.copy(out=tmp, in_=avs[1])
                        else:
                            nc.vector.tensor_copy(out=tmp, in_=avs[1])
                        for hh, src in ((0, dr[0:1, :]),
                                        (1, tmp[HD:HD + 1, :])):
                            beng.dma_start(
                                out=den[hh * RPH:(hh + 1) * RPH, q4, :],
                                in_=src)

                        def normalize(q4=q4, q0=q0, den=den, recl=recl,
                                      tmp=tmp, pr=pr, beng=beng,
                                      last_pair=last_pair):
                            dsl = den[:, q4, :]
                            with nc.allow_low_precision(
                                    reason="bf16 softmax denominator"):
                                nc.vector.reciprocal(out=dsl, in_=dsl)
                            for hh in (0, 1):
                                beng.dma_start(
                                    out=recl[hh:hh + 1, q4, :],
                                    in_=den[hh * RPH:(hh + 1) * RPH, q4, :])
                            if last_pair and q4 == NQ - 1:
                                bps = av_ps.tile([P, QS], F32,
                                                 name=f"bps_{pr}_{q4}",
                                                 tag="av")
                            else:
                                bps = mm_ps.tile([P, QS], F32,
                                                 name=f"bps_{pr}_{q4}",
                                                 tag="mm")
                            nc.tensor.matmul(
                                bps, lhsT=sel2_sb, rhs=recl[:, q4, :],
                                start=True, stop=True)
                            # odd-head mul + writeback first: the attn
                            # bounce is on the critical path of the out
                            # chains; the even-half mul is not
                            nc.vector.tensor_mul(out=tmp[0:HD, :],
                                                 in0=tmp[0:HD, :],
                                                 in1=bps[HD:P, :])
                            beng.dma_start(
                                out=attn_sb[HD:P, pr, q0:q0 + QS],
                                in_=tmp[0:HD, :])
                            sl = attn_sb[0:HD, pr, q0:q0 + QS]
                            nc.vector.tensor_mul(out=sl, in0=sl,
                                                 in1=bps[0:HD, :])
                            # unlock this chunk's out chains, due over the
                            # following ticks (leave a few groups of slack
                            # after this normalize's attn writeback)
                            if last_pair and q4 < NQ - 1:
                                i = 0
                                for qt in range(q4 * (QS // P),
                                                (q4 + 1) * (QS // P)):
                                    for f0 in range(0, D, 512):
                                        for p_ in out_chain_parts(
                                                qt, f0, min(512, D - f0)):
                                            fillers.push(
                                                p_,
                                                deadline=fillers.ticks
                                                + 6 + i // 2)
                                            i += 1

                        pending_norm.append(normalize)

                    if HOIST_CLOSE:
                        pending_close.append(close)
                    else:
                        close()

            for pr in range(PAIRS):
                if pr == PAIRS - 1:
                    fillers.drain()
                    fillers.emitted = 0
                    fillers.ticks = 0
                    fillers.rate = 0.0
                attention_pair(pr, pr == PAIRS - 1)
            fillers.drain()
            # tail: the final chunk's close + normalize, with its out
            # chains j-granular so the j<3 matmuls (pairs 0-2, already
            # normalized) overlap the final normalize's latency chain
            q4t = NQ - 1
            tail_specs = [(qt, f0)
                          for qt in range(q4t * (QS // P),
                                          (q4t + 1) * (QS // P))
                          for f0 in range(0, D, 512)]
            tail_parts = [
                out_chain_parts(qt, f0, min(512, D - f0), nsplit=4,
                                pool=(st_ps if k % 2 else mm_ps),
                                ptag=("st" if k % 2 else "mm"))
                for k, (qt, f0) in enumerate(tail_specs)]
            while pending_close:
                pending_close.popleft()()
            for k in range(4):
                tail_parts[k][0]()
                tail_parts[k][1]()
            for k in range(4):
                tail_parts[k][2]()
            while pending_norm:
                pending_norm.popleft()()
            for k in range(4):
                tail_parts[k][3]()
                tail_parts[k + 4][0]()
            for k in range(4, 8):
                tail_parts[k][1]()
                tail_parts[k][2]()
            for k in range(4, 8):
                tail_parts[k][3]()

    nc.compile()
    return nc


def to_bf16(a):
    import ml_dtypes
    return np.ascontiguousarray(a).astype(ml_dtypes.bfloat16)


def make_core_inputs(x, Wqkv_w, Wqkv_b, out_w, out_b, H, n_tp):
    """Host-side shard + layout prep. Returns list of in_maps (one per core).
    Core c handles batch c // n_tp, head group c % n_tp."""
    B, L, D = x.shape
    hpg = H // n_tp            # heads per core
    PAIRS = hpg // 2
    EQK = 2 * hpg * HD
    EV = hpg * HD
    ET = EQK // P
    DBLK = D // P
    tri = np.triu(np.ones((P, P), dtype=np.float32))  # [k, q]: 1 if q >= k
    in_maps = []
    for c in range(B * n_tp):
        b, g = c // n_tp, c % n_tp
        # qk row order: per pair p -> q(2p), q(2p+1), k(2p), k(2p+1)
        qk_rows = []
        for p_ in range(PAIRS):
            for h in (2 * p_, 2 * p_ + 1):
                qk_rows.extend(range(g * hpg * HD + h * HD,
                                     g * hpg * HD + h * HD + HD))
            for h in (2 * p_, 2 * p_ + 1):
                qk_rows.extend(range(D + g * hpg * HD + h * HD,
                                     D + g * hpg * HD + h * HD + HD))
        v_rows = list(range(2 * D + g * hpg * HD, 2 * D + (g + 1) * hpg * HD))
        # wqkT host layout: [P, ET, DBLK, P]:
        #   wqkT[p, et, o, c] = Wqkv_w[qk_rows[et*128 + c], o*128 + p]
        wqk_t = Wqkv_w[np.array(qk_rows)].T            # [D, EQK]
        wqk_t = wqk_t.reshape(DBLK, P, ET, P)          # [o, p, et, c]
        wqk_t = np.transpose(wqk_t, (1, 2, 0, 3))      # [p, et, o, c]
        # wvT host layout: [P, DBLK, EV]
        wv_t = Wqkv_w[np.array(v_rows)].T.reshape(DBLK, P, EV)
        wv_t = np.transpose(wv_t, (1, 0, 2))
        in_maps.append({
            "xT": to_bf16(x[b].T),
            "wqkT": to_bf16(wqk_t),
            "wvT": to_bf16(wv_t),
            "bqk": np.ascontiguousarray(
                Wqkv_b[np.array(qk_rows)].reshape(ET, P).T),
            "vb": to_bf16(np.tile(Wqkv_b[np.array(v_rows)], (P, 1))),
            "woT": to_bf16(out_w[:, g * EV:(g + 1) * EV].T),
            "ob": to_bf16(np.tile(out_b, (P, 1)) if g == 0
                          else np.zeros((P, D), np.float32)),
            "tri": to_bf16(tri),
            "onep": to_bf16(np.ones((P, HD), np.float32)),
            "sel2": to_bf16(np.concatenate([
                np.concatenate([np.ones((1, HD)), np.zeros((1, HD))], 1),
                np.concatenate([np.zeros((1, HD)), np.ones((1, HD))], 1),
            ]).astype(np.float32)),
            "onec": to_bf16(np.ones((P, L // P), np.float32)),
        })
    return in_maps


_NC_CACHE = {}
LAST_RESULTS = None


def kernel(x, Wqkv_w, Wqkv_b, out_w, out_b):
    global LAST_RESULTS
    x = np.asarray(x, dtype=np.float32)
    Wqkv_w = np.asarray(Wqkv_w, dtype=np.float32)
    Wqkv_b = np.asarray(Wqkv_b, dtype=np.float32)
    out_w = np.asarray(out_w, dtype=np.float32)
    out_b = np.asarray(out_b, dtype=np.float32)

    B, L, D = x.shape
    H = 16
    n_tp = 2
    hpg = H // n_tp

    key = (L, D, hpg)
    if key not in _NC_CACHE:
        _NC_CACHE[key] = build_mha_nc(L, D, hpg)
    nc = _NC_CACHE[key]

    in_maps = make_core_inputs(x, Wqkv_w, Wqkv_b, out_w, out_b, H, n_tp)

    from concourse.bass_utils import run_bass_kernel_spmd
    res = run_bass_kernel_spmd(nc, in_maps, core_ids=list(range(len(in_maps))))
    LAST_RESULTS = res

    out = np.empty((B, L, D), dtype=np.float32)
    for b in range(B):
        out[b] = np.asarray(res.results[n_tp * b]["out"], dtype=np.float32)
        for g in range(1, n_tp):
            out[b] += np.asarray(res.results[n_tp * b + g]["out"],
                                 dtype=np.float32)
    return out


if __name__ == "__main__":
    nc = build_mha_nc(2048, 1024, 8)
    print("built OK")
